# revision 3
# baseline (speedup 1.0000x reference)
"""Trainium2 Bass kernel for nn_BatchFrechetMean: recursive weighted Frechet mean
of SPD matrices under the affine-invariant metric.

Algorithm (eigh-free, GEMM-only, all-fp32r):
  * Factored state: Ct (=C^T with M = C C^T), Z (=C^{-1}), Zt (=Z^T).
    Step:  S = Z f Z^T;  Ct' = E+ Ct;  Z' = E- Z;  Zt' = Z^T E-,
    with E+- = S^{+-t/2} = exp(+-X), X = (t/2) log S.
  * log S: degree-7 Chebyshev fit on [0.33, 5.2] in u = al*S + be*I, split as
    p(u) = a0 + a1 T1 + a2 T2 + T4*(b0 + b1 T1 + b2 T2) + T3*(c3 + c7 T4),
    evaluated with 4 GEMMs (u^2, T2@u, T2@T2, final accumulate).  exp:
    order-3 Taylor (E+- = I + X2/2 +- X(I + X2/6)).
  * All matmuls are float32r (1 cyc/row vs 4 for fp32; ~12-bit mantissa).
    Noise control: state updates in DELTA form - D+- = E+- - I is staged
    (small entries -> small absolute rounding), the identity term is added
    exactly into the PSUM via an identity-lhsT matmul ("fold").  Scalar
    multiples of I are folded the same way, freeing the vector engines.
  * Parallelism: the geodesic map is contractive, so 16 independent windows
    of W=8 warmup + 16 kept steps; 2 windows (chains) per core with LOCKSTEP
    interleaved emission so the in-order engine queues alternate chains and
    one chain's dependency gaps are covered by the other's work.
  * Engine balance: PSUM->SBUF stagings split across ACT (scaled copies) and
    DVE (scalar_tensor_tensor); Pool takes an SBUF-only add.  Stagings are
    emitted in halves over per-m-block split PSUM tiles so consumer GEMMs
    start after the first half.

Matrix layout: 256x256 matrix X as one [128, 512] SBUF tile,
tile[p, b*256 + j] = X[b*128 + p, j].  GEMM out = lhsT^T @ rhs is 4 matmuls
(2 m-blocks x 2 k-blocks); identity folds add 1 matmul per m-block.
"""
import numpy as np

import concourse.bacc as bacc
import concourse.mybir as mybir
from concourse.tile import TileContext
from concourse.bass_utils import run_bass_kernel_spmd

P = 128
N = 256
B = 256
NCORES = 8
NWIN = 16            # windows total (2 per core, interleaved)
L_KEEP = B // NWIN   # 16 kept steps per window
W_WARM = 7
NSTEP = W_WARM + L_KEEP
CHEB_A, CHEB_B = 0.33, 5.20
CHEB_DEG = 7

F32 = mybir.dt.float32
F32R = mybir.dt.float32r
ALU = mybir.AluOpType
ACTF = mybir.ActivationFunctionType


# ----------------------------- host helpers -----------------------------

def to_tile(x):
    """256x256 -> [128,512] tile layout."""
    return np.ascontiguousarray(
        x.reshape(2, P, N).transpose(1, 0, 2).reshape(P, 2 * N))


def from_tile(x):
    return np.ascontiguousarray(
        x.reshape(P, 2, N).transpose(1, 0, 2).reshape(N, N))


def round_f32r(x):
    """Round to the fp32r grid (11 explicit mantissa bits) so DMA-fed matmul
    operands match what the PE expects for float32r inputs."""
    m, ex = np.frexp(np.asarray(x, np.float64))
    s = 2.0 ** 12
    return np.ldexp(np.round(m * s) / s, ex).astype(np.float32)


def cheb_log_coeffs(a, b, deg):
    M = 2000
    u = np.cos((2 * np.arange(M) + 1) * np.pi / (2 * M))
    x = 0.5 * (b - a) * u + 0.5 * (b + a)
    V = np.polynomial.chebyshev.chebvander(u, deg)
    coef, *_ = np.linalg.lstsq(V, np.log(x), rcond=None)
    return coef


def ps_chunks_u(c):
    """deg-7 cheb coeffs -> exact decomposition
    p(u) = a0 + a1 T1 + a2 T2  +  T4*(b0 + b1 T1 + b2 T2)  +  T3*(c3 + c7 T4).
    The T3 factor is applied as an extra accumulated GEMM T3 @ (c3 I + c7 T4)
    so the H1 tile build does not wait on T3.  Returns (a[3], b[3], c3, c7)."""
    from numpy.polynomial import chebyshev as Ch

    def basis_poly(*ks):
        bp = np.array([1.0])
        for k in ks:
            t = np.zeros(k + 1)
            t[k] = 1.0
            bp = Ch.chebmul(bp, t)
        return np.pad(bp, (0, 12 - len(bp)))

    cols = [basis_poly(0), basis_poly(1), basis_poly(2),
            basis_poly(4), basis_poly(4, 1), basis_poly(4, 2),
            basis_poly(3), basis_poly(3, 4)]
    Bm = np.array(cols).T
    target = np.pad(c, (0, Bm.shape[0] - len(c)))
    fcs, *_ = np.linalg.lstsq(Bm, target, rcond=None)
    assert np.linalg.norm(Bm @ fcs - target) < 1e-9
    return fcs[0:3], fcs[3:6], float(fcs[6]), float(fcs[7])


# ----------------------------- device program -----------------------------
# A GEMM is out = lhsT^T @ rhs on 256x256 matrices in tile layout: 2 output
# m-blocks x 2 contraction k-blocks.  Each m-block accumulates in its OWN
# [P, N] psum tile so its staging can start after its own matmuls.  "folds"
# add diag_const^T @ rhs (one matmul per m-block, diagonal k=m block only)
# and are emitted first since their operands are ready earlier.

def emit_gemm_sp(nc, ps, name, mains, folds=()):
    pab = []
    for m in range(2):
        pm = ps.tile([P, N], F32, tag="ps", name=f"{name}m{m}")
        msl = slice(m * N, (m + 1) * N)
        total = len(folds) + 2 * len(mains)
        i = 0
        for fl, fr in folds:
            nc.tensor.matmul(pm[:, :],
                             fl[:, m * N + m * P: m * N + m * P + P],
                             fr[:, msl],
                             start=(i == 0), stop=(i == total - 1))
            i += 1
        for lhsT, rhs in mains:
            for k in range(2):
                nc.tensor.matmul(pm[:, :],
                                 lhsT[:, k * N + m * P: k * N + m * P + P],
                                 rhs[:, k * N:(k + 1) * N],
                                 start=(i == 0), stop=(i == total - 1))
                i += 1
        pab.append(pm)
    return pab


def emit_gemm_1(nc, ps, name, mains, folds=()):
    """Like emit_gemm_sp but both m-blocks in ONE [P, 2N] psum tile (1 bank):
    staging is a single cheaper instruction; use for off-path GEMMs."""
    pm = ps.tile([P, 2 * N], F32, tag="ps2", name=name)
    total = 2 * (len(folds) + 2 * len(mains))
    i = 0
    for m in range(2):
        msl = slice(m * N, (m + 1) * N)
        for fl, fr in folds:
            nc.tensor.matmul(pm[:, msl],
                             fl[:, m * N + m * P: m * N + m * P + P],
                             fr[:, msl],
                             start=(i == 0), stop=(i == total - 1))
            i += 1
        for lhsT, rhs in mains:
            for k in range(2):
                nc.tensor.matmul(pm[:, msl],
                                 lhsT[:, k * N + m * P: k * N + m * P + P],
                                 rhs[:, k * N:(k + 1) * N],
                                 start=(i == 0), stop=(i == total - 1))
                i += 1
    return pm


def build_program():
    c = cheb_log_coeffs(CHEB_A, CHEB_B, CHEB_DEG)
    ca_, cb_, c3, c7 = ps_chunks_u(c)
    al = 2.0 / (CHEB_B - CHEB_A)
    be = -(CHEB_B + CHEB_A) / (CHEB_B - CHEB_A)

    # gamma-shift: evaluate (T4 + g I)@H1 with g = a2/b2 so the a2*T2 fold
    # vanishes: p = T4g@H1 + T3@R37 + a0' I + a1' u  with ai' = ai - g*bi.
    g = float(ca_[2] / cb_[2])
    a0p = float(ca_[0] - g * cb_[0])
    a1p = float(ca_[1] - g * cb_[1])
    kap = (g - 1.0) / 2.0             # pT4 psum = T2@T2 + kap*I; T4g = 2*psum

    iden = np.eye(N, dtype=np.float64)
    consts = {
        "iden": iden,
        "ibe2": be * iden,            # u = al*psum + ibe2
        "ikap": kap * iden,           # fold for T4g
        "B0c": cb_[0] * iden,         # H1a = b1*u + B0c
        "A0c": a0p * iden,            # pL0 folds
        "A1c": a1p * iden,
        "R0c": (c3 - c7 * g) * iden,  # R37 = 2*c7*pT4' + (c3 - c7*g)*I
    }
    CONST_NAMES = list(consts)
    const_arr = np.concatenate(
        [to_tile(round_f32r(consts[k])) for k in CONST_NAMES], axis=1)
    b1, b2 = float(cb_[1]), float(cb_[2])

    nc = bacc.Bacc()
    f_in = nc.declare_dram_parameter("fs", [2, NSTEP, P, 2 * N], F32,
                                     isOutput=False)
    tv_in = nc.declare_dram_parameter("tv", [P, 2 * NSTEP], F32, isOutput=False)
    c_in = nc.declare_dram_parameter("consts", [P, 2 * N * len(CONST_NAMES)],
                                     F32, isOutput=False)
    m_out = nc.declare_dram_parameter("means", [2, L_KEEP, P, 2 * N], F32,
                                      isOutput=True)

    with TileContext(nc) as tc:
        with (
            tc.tile_pool(name="consts", bufs=1) as cpool,
            tc.tile_pool(name="state", bufs=4) as spool,
            tc.tile_pool(name="work", bufs=2) as wpool,
            tc.tile_pool(name="fin", bufs=4) as fpool,
            tc.tile_pool(name="mout", bufs=2) as opool,
            tc.tile_pool(name="ps", bufs=6, space="PSUM") as ps,
            tc.tile_pool(name="ps2", bufs=2, space="PSUM") as ps2,
        ):
            CT = cpool.tile([P, 2 * N * len(CONST_NAMES)], F32R, tag="cc")
            nc.sync.dma_start(CT[:, :], c_in[:, :].bitcast(F32R))
            cv = {k: CT[:, i * 2 * N:(i + 1) * 2 * N]
                  for i, k in enumerate(CONST_NAMES)}
            TV = cpool.tile([P, 2 * NSTEP], F32, tag="tv")
            nc.sync.dma_start(TV[:, :], tv_in[:, :])

            IDEN = cv["iden"]
            Zt = [None, None]
            Z = [None, None]
            Ct = [None, None]

            def st(x):
                return IDEN if x is None else x[:, :]

            def act_halves(out, pab, scale=1.0):
                """scaled copy of a split-psum pair into tile halves (ACT)."""
                for h in range(2):
                    nc.scalar.activation(out[:, h * N:(h + 1) * N],
                                         pab[h][:, :], ACTF.Copy, scale=scale)

            def dve_halves(out, pab, scalar, in1, op0, op1):
                for h in range(2):
                    nc.vector.scalar_tensor_tensor(
                        out[:, h * N:(h + 1) * N], pab[h][:, :], scalar,
                        in1[:, h * N:(h + 1) * N], op0=op0, op1=op1)

            def act_full(out, pab, scale=1.0):
                """single full-width scaled copy over a split-psum pair
                (the pair is two ring-adjacent banks; read each separately
                but in ONE instruction is not expressible - use two, so
                "full" here just means emitted as the cheaper unsplit pair
                order for off-path results)."""
                act_halves(out, pab, scale=scale)

            def dve_full(out, pab, scalar, in1, op0, op1):
                dve_halves(out, pab, scalar, in1, op0, op1)

            # Lockstep emission: for each step j, every op is emitted for
            # chain 0 then chain 1, so the in-order engine queues alternate
            # chains and one chain's dependency gaps are covered by the
            # other's work.
            def step_ops(j):
                V = [{}, {}]

                def w_tile(ch, nm, dt=F32R):
                    V[ch][nm] = wpool.tile([P, 2 * N], dt, tag=f"{nm}{ch}",
                                           name=f"{nm}{ch}_{j}")
                    return V[ch][nm]

                def op_f(ch, v):
                    v["fs"] = fpool.tile([P, 2 * N], F32R, tag=f"f{ch}",
                                         name=f"fs{ch}_{j}")
                    nc.sync.dma_start(v["fs"][:, :],
                                      f_in[ch, j, :, :].bitcast(F32R))

                def op_pW(ch, v):
                    v["pW"] = emit_gemm_sp(nc, ps, f"pW{ch}_{j}",
                                           [(v["fs"][:, :], st(Zt[ch]))])

                def op_Wt(ch, v):
                    act_halves(w_tile(ch, "Wt"), v["pW"])

                def op_pS(ch, v):
                    v["pS"] = emit_gemm_sp(nc, ps, f"pS{ch}_{j}",
                                           [(st(Zt[ch]), v["Wt"][:, :])])

                def op_u(ch, v):
                    dve_halves(w_tile(ch, "u"), v["pS"], float(al),
                               cv["ibe2"], ALU.mult, ALU.add)

                def op_pU2(ch, v):
                    v["pU2"] = emit_gemm_sp(nc, ps, f"pU2{ch}_{j}",
                                            [(v["u"][:, :], v["u"][:, :])])

                def op_T2(ch, v):
                    dve_halves(w_tile(ch, "T2"), v["pU2"], 2.0, IDEN,
                               ALU.mult, ALU.subtract)

                def op_pT3(ch, v):
                    v["pT3"] = emit_gemm_sp(nc, ps, f"pT3{ch}_{j}",
                                            [(v["T2"][:, :], v["u"][:, :])])

                def op_T3(ch, v):
                    dve_halves(w_tile(ch, "T3"), v["pT3"], 2.0, v["u"][:, :],
                               ALU.mult, ALU.subtract)

                def op_pT4(ch, v):
                    # psum' = T2@T2 + kap I;  T4g = 2*psum' (ACT), and
                    # R37 = 2*c7*psum' + (c3 - c7*g) I (DVE)
                    v["pT4"] = emit_gemm_1(nc, ps2, f"pT4{ch}_{j}",
                                           [(v["T2"][:, :], v["T2"][:, :])],
                                           folds=[(cv["ikap"], IDEN)])

                def op_T4(ch, v):
                    t = w_tile(ch, "T4")
                    nc.scalar.activation(t[:, :], v["pT4"][:, :], ACTF.Copy,
                                         scale=2.0)

                def op_R37(ch, v):
                    t = w_tile(ch, "R37")
                    nc.vector.scalar_tensor_tensor(
                        t[:, :], v["pT4"][:, :], 2.0 * c7, cv["R0c"],
                        op0=ALU.mult, op1=ALU.add)

                def op_H1a(ch, v):
                    t = w_tile(ch, "H1a")
                    nc.vector.scalar_tensor_tensor(
                        t[:, :], v["u"][:, :], b1, cv["B0c"],
                        op0=ALU.mult, op1=ALU.add)

                def op_H1(ch, v):
                    t = w_tile(ch, "H1")
                    nc.vector.scalar_tensor_tensor(
                        t[:, :], v["T2"][:, :], b2, v["H1a"][:, :],
                        op0=ALU.mult, op1=ALU.add)

                def op_pL0(ch, v):
                    v["pL0"] = emit_gemm_sp(
                        nc, ps, f"pL0{ch}_{j}",
                        [(v["T3"][:, :], v["R37"][:, :]),
                         (v["T4"][:, :], v["H1"][:, :])],
                        folds=[(cv["A0c"], IDEN), (cv["A1c"], v["u"][:, :])])

                def op_X(ch, v):
                    tvc = TV[:, ch * NSTEP + j: ch * NSTEP + j + 1]
                    act_halves(w_tile(ch, "X"), v["pL0"], scale=tvc)

                def op_pX2(ch, v):
                    v["pX2"] = emit_gemm_sp(nc, ps, f"pX2{ch}_{j}",
                                            [(v["X"][:, :], v["X"][:, :])])

                def op_Shi(ch, v):
                    dve_halves(w_tile(ch, "Shi"), v["pX2"], float(1 / 6),
                               IDEN, ALU.mult, ALU.add)

                def op_Chm(ch, v):
                    act_halves(w_tile(ch, "Chm"), v["pX2"], scale=0.5)

                def op_pSh(ch, v):
                    v["pSh"] = emit_gemm_sp(nc, ps, f"pSh{ch}_{j}",
                                            [(v["X"][:, :], v["Shi"][:, :])])

                def op_Dm(ch, v):
                    # on DVE straight from PSUM: next step's Zt needs it
                    dve_halves(w_tile(ch, "Dm"), v["pSh"], -1.0,
                               v["Chm"][:, :], ALU.mult, ALU.add)

                def op_Dp(ch, v):
                    # Dp = Sh + Chm = 2*Chm - Dm (no Sh staging needed)
                    t = w_tile(ch, "Dp")
                    nc.vector.scalar_tensor_tensor(
                        t[:, :], v["Chm"][:, :], 2.0, v["Dm"][:, :],
                        op0=ALU.mult, op1=ALU.subtract)

                def op_pZt(ch, v):
                    v["pZt"] = emit_gemm_sp(nc, ps, f"pZt{ch}_{j}",
                                            [(st(Z[ch]), v["Dm"][:, :])],
                                            folds=[(IDEN, st(Zt[ch]))])

                def op_Ztn(ch, v):
                    v["Ztn"] = spool.tile([P, 2 * N], F32R, tag=f"Zt{ch}",
                                          name=f"Ztn{ch}_{j}")
                    act_halves(v["Ztn"], v["pZt"])

                def op_pZ(ch, v):
                    v["pZ"] = emit_gemm_1(nc, ps2, f"pZ{ch}_{j}",
                                          [(v["Dm"][:, :], st(Z[ch]))],
                                          folds=[(IDEN, st(Z[ch]))])

                def op_Zn(ch, v):
                    v["Zn"] = spool.tile([P, 2 * N], F32R, tag=f"Z{ch}",
                                         name=f"Zn{ch}_{j}")
                    nc.scalar.activation(v["Zn"][:, :], v["pZ"][:, :],
                                         ACTF.Copy)

                def op_pCt(ch, v):
                    v["pCt"] = emit_gemm_1(nc, ps2, f"pCt{ch}_{j}",
                                           [(v["Dp"][:, :], st(Ct[ch]))],
                                           folds=[(IDEN, st(Ct[ch]))])

                def op_Ctn(ch, v):
                    v["Ctn"] = spool.tile([P, 2 * N], F32R, tag=f"Ct{ch}",
                                          name=f"Ctn{ch}_{j}")
                    nc.scalar.activation(v["Ctn"][:, :], v["pCt"][:, :],
                                         ACTF.Copy)

                def op_out_prev(ch, v):
                    # output of step j-1 (committed state), deferred here so
                    # its GEMM fills gaps instead of stalling the PE tail
                    if j - 1 < W_WARM or Ct[ch] is None:
                        return
                    pM = emit_gemm_1(nc, ps2, f"pM{ch}_{j - 1}",
                                     [(st(Ct[ch]), st(Ct[ch]))])
                    Mo = opool.tile([P, 2 * N], F32, tag=f"Mo{ch}",
                                    name=f"Mo{ch}_{j - 1}")
                    nc.scalar.activation(Mo[:, :], pM[:, :], ACTF.Copy)
                    nc.sync.dma_start(m_out[ch, j - 1 - W_WARM, :, :],
                                      Mo[:, :])

                OPS1 = (op_f, op_out_prev, op_pW, op_Wt, op_pS, op_u,
                        op_pU2, op_T2, op_pT3, op_T3, op_pT4, op_T4,
                        op_R37, op_H1a, op_H1, op_pL0, op_X)
                OPS2 = (op_pX2, op_Shi, op_Chm, op_pSh, op_Dm,
                        op_Dp, op_pZt, op_Ztn, op_pZ, op_Zn, op_pCt,
                        op_Ctn)
                yield OPS1, OPS2, V


            # phase-shifted 2-chain schedule: per j emit
            #   A-phase1(j), B-phase2(j-1), B-phase1(j), A-phase2(j)
            # so each chain's mid-step dependency gap is covered by the
            # other chain's independent work.
            gens = {}
            phases = {}
            commit = {}

            def start(jj):
                gen = step_ops(jj)
                o1, o2, vv = next(gen)
                gens[jj] = (o1, o2, vv)

            def zip_phases(a_ops, a_v, b_ops, b_v):
                na, nb = len(a_ops or ()), len(b_ops or ())
                for i in range(max(na, nb)):
                    if i < na:
                        a_ops[i](0, a_v)
                    if i < nb:
                        b_ops[i](1, b_v)

            def commit(jj, ch):
                vv = gens[jj][2][ch]
                Zt[ch], Z[ch], Ct[ch] = vv["Ztn"], vv["Zn"], vv["Ctn"]

            for j in range(NSTEP + 1):
                if j < NSTEP:
                    start(j)
                ph_a1 = gens[j][0] if j < NSTEP else None
                av = gens[j][2][0] if j < NSTEP else None
                ph_b2 = gens[j - 1][1] if j >= 1 else None
                bv = gens[j - 1][2][1] if j >= 1 else None
                zip_phases(ph_a1 or (), av, ph_b2 or (), bv)
                if j >= 1:
                    commit(j - 1, 1)
                if j < NSTEP:
                    ph_a2, vv = gens[j][1], gens[j][2]
                    ph_b1 = gens[j][0]
                    na, nb = len(ph_a2), len(ph_b1)
                    for i in range(max(na, nb)):
                        if i < na:
                            ph_a2[i](0, vv[0])
                        if i < nb:
                            ph_b1[i](1, vv[1])
                    commit(j, 0)
                if j >= 1:
                    del gens[j - 1]

            # final kept outputs (state of the last step, never emitted by
            # the deferred op_out_prev)
            for ch in (0, 1):
                pM = emit_gemm_1(nc, ps2, f"pMfin{ch}",
                                 [(Ct[ch][:, :], Ct[ch][:, :])])
                Mo = opool.tile([P, 2 * N], F32, tag=f"Mo{ch}",
                                name=f"Mofin{ch}")
                nc.scalar.activation(Mo[:, :], pM[:, :], ACTF.Copy)
                nc.sync.dma_start(m_out[ch, L_KEEP - 1, :, :], Mo[:, :])

    nc.compile()
    return nc, const_arr


_CACHED = {}


def prep_inputs(f, weights, const_arr):
    f = np.asarray(f, dtype=np.float32)
    weights = np.asarray(weights, dtype=np.float32)
    fs = f[:, 0]
    e = np.exp(weights - weights.max(axis=1, keepdims=True))
    t = (e / e.sum(axis=1, keepdims=True))[:, 1].astype(np.float32)

    iden = np.eye(N, dtype=np.float32)
    f_tiles = np.empty((B + W_WARM, P, 2 * N), np.float32)
    f_tiles[:W_WARM] = to_tile(iden)
    for k in range(B):
        f_tiles[W_WARM + k] = to_tile(round_f32r(fs[k]))
    t_pad = np.concatenate([np.zeros(W_WARM, np.float32), t])

    in_maps = []
    for c in range(NCORES):
        fa = np.empty((2, NSTEP, P, 2 * N), np.float32)
        tva = np.empty((P, 2 * NSTEP), np.float32)
        for ch in range(2):
            w = 2 * c + ch
            s0 = w * L_KEEP           # window start in padded idx
            fa[ch] = f_tiles[s0:s0 + NSTEP]
            tva[:, ch * NSTEP:(ch + 1) * NSTEP] = np.broadcast_to(
                0.5 * t_pad[s0:s0 + NSTEP], (P, NSTEP))
        in_maps.append({"fs": np.ascontiguousarray(fa),
                        "tv": np.ascontiguousarray(tva),
                        "consts": const_arr})
    return in_maps


def unpack_outputs(results):
    out = np.empty((B, N, N), np.float32)
    for c in range(NCORES):
        m = results[c]["means"]       # [2, L_KEEP, P, 2N]
        for ch in range(2):
            w = 2 * c + ch
            for j in range(L_KEEP):
                out[w * L_KEEP + j] = from_tile(m[ch, j])
    return out[:, None]


def kernel(f, weights):
    if "prog" not in _CACHED:
        _CACHED["prog"] = build_program()
    nc, const_arr = _CACHED["prog"]
    in_maps = prep_inputs(f, weights, const_arr)
    res = run_bass_kernel_spmd(nc, in_maps, list(range(NCORES)))
    return unpack_outputs(res.results)


# revision 4
# speedup vs baseline: 1.0035x; 1.0035x over previous
"""Trainium2 Bass kernel for nn_BatchFrechetMean: recursive weighted Frechet mean
of SPD matrices under the affine-invariant metric.

Algorithm (eigh-free, GEMM-only, all-fp32r):
  * Factored state: Ct (=C^T with M = C C^T), Z (=C^{-1}), Zt (=Z^T).
    Step:  S = Z f Z^T;  Ct' = E+ Ct;  Z' = E- Z;  Zt' = Z^T E-,
    with E+- = S^{+-t/2} = exp(+-X), X = (t/2) log S.
  * log S: degree-7 Chebyshev fit on [0.33, 5.2] in u = al*S + be*I, split as
    p(u) = a0 + a1 T1 + a2 T2 + T4*(b0 + b1 T1 + b2 T2) + T3*(c3 + c7 T4),
    evaluated with 4 GEMMs (u^2, T2@u, T2@T2, final accumulate).  exp:
    order-3 Taylor (E+- = I + X2/2 +- X(I + X2/6)).
  * All matmuls are float32r (1 cyc/row vs 4 for fp32; ~12-bit mantissa).
    Noise control: state updates in DELTA form - D+- = E+- - I is staged
    (small entries -> small absolute rounding), the identity term is added
    exactly into the PSUM via an identity-lhsT matmul ("fold").  Scalar
    multiples of I are folded the same way, freeing the vector engines.
  * Parallelism: the geodesic map is contractive, so 16 independent windows
    of W=8 warmup + 16 kept steps; 2 windows (chains) per core with LOCKSTEP
    interleaved emission so the in-order engine queues alternate chains and
    one chain's dependency gaps are covered by the other's work.
  * Engine balance: PSUM->SBUF stagings split across ACT (scaled copies) and
    DVE (scalar_tensor_tensor); Pool takes an SBUF-only add.  Stagings are
    emitted in halves over per-m-block split PSUM tiles so consumer GEMMs
    start after the first half.

Matrix layout: 256x256 matrix X as one [128, 512] SBUF tile,
tile[p, b*256 + j] = X[b*128 + p, j].  GEMM out = lhsT^T @ rhs is 4 matmuls
(2 m-blocks x 2 k-blocks); identity folds add 1 matmul per m-block.
"""
import numpy as np

import concourse.bacc as bacc
import concourse.mybir as mybir
from concourse.tile import TileContext
from concourse.bass_utils import run_bass_kernel_spmd

P = 128
N = 256
B = 256
NCORES = 8
NWIN = 16            # windows total (2 per core, interleaved)
L_KEEP = B // NWIN   # 16 kept steps per window
W_WARM = 7
NSTEP = W_WARM + L_KEEP
CHEB_A, CHEB_B = 0.33, 5.20
CHEB_DEG = 7

F32 = mybir.dt.float32
F32R = mybir.dt.float32r
ALU = mybir.AluOpType
ACTF = mybir.ActivationFunctionType


# ----------------------------- host helpers -----------------------------

def to_tile(x):
    """256x256 -> [128,512] tile layout."""
    return np.ascontiguousarray(
        x.reshape(2, P, N).transpose(1, 0, 2).reshape(P, 2 * N))


def from_tile(x):
    return np.ascontiguousarray(
        x.reshape(P, 2, N).transpose(1, 0, 2).reshape(N, N))


def round_f32r(x):
    """Round to the fp32r grid (11 explicit mantissa bits) so DMA-fed matmul
    operands match what the PE expects for float32r inputs."""
    m, ex = np.frexp(np.asarray(x, np.float64))
    s = 2.0 ** 12
    return np.ldexp(np.round(m * s) / s, ex).astype(np.float32)


def cheb_log_coeffs(a, b, deg):
    M = 2000
    u = np.cos((2 * np.arange(M) + 1) * np.pi / (2 * M))
    x = 0.5 * (b - a) * u + 0.5 * (b + a)
    V = np.polynomial.chebyshev.chebvander(u, deg)
    coef, *_ = np.linalg.lstsq(V, np.log(x), rcond=None)
    return coef


def ps_chunks_u(c):
    """deg-7 cheb coeffs -> exact decomposition
    p(u) = a0 + a1 T1 + a2 T2  +  T4*(b0 + b1 T1 + b2 T2)  +  T3*(c3 + c7 T4).
    The T3 factor is applied as an extra accumulated GEMM T3 @ (c3 I + c7 T4)
    so the H1 tile build does not wait on T3.  Returns (a[3], b[3], c3, c7)."""
    from numpy.polynomial import chebyshev as Ch

    def basis_poly(*ks):
        bp = np.array([1.0])
        for k in ks:
            t = np.zeros(k + 1)
            t[k] = 1.0
            bp = Ch.chebmul(bp, t)
        return np.pad(bp, (0, 12 - len(bp)))

    cols = [basis_poly(0), basis_poly(1), basis_poly(2),
            basis_poly(4), basis_poly(4, 1), basis_poly(4, 2),
            basis_poly(3), basis_poly(3, 4)]
    Bm = np.array(cols).T
    target = np.pad(c, (0, Bm.shape[0] - len(c)))
    fcs, *_ = np.linalg.lstsq(Bm, target, rcond=None)
    assert np.linalg.norm(Bm @ fcs - target) < 1e-9
    return fcs[0:3], fcs[3:6], float(fcs[6]), float(fcs[7])


# ----------------------------- device program -----------------------------
# A GEMM is out = lhsT^T @ rhs on 256x256 matrices in tile layout: 2 output
# m-blocks x 2 contraction k-blocks.  Each m-block accumulates in its OWN
# [P, N] psum tile so its staging can start after its own matmuls.  "folds"
# add diag_const^T @ rhs (one matmul per m-block, diagonal k=m block only)
# and are emitted first since their operands are ready earlier.

def emit_gemm_sp(nc, ps, name, mains, folds=()):
    pab = []
    for m in range(2):
        pm = ps.tile([P, N], F32, tag="ps", name=f"{name}m{m}")
        msl = slice(m * N, (m + 1) * N)
        total = len(folds) + 2 * len(mains)
        i = 0
        for fl, fr in folds:
            nc.tensor.matmul(pm[:, :],
                             fl[:, m * N + m * P: m * N + m * P + P],
                             fr[:, msl],
                             start=(i == 0), stop=(i == total - 1))
            i += 1
        for lhsT, rhs in mains:
            for k in range(2):
                nc.tensor.matmul(pm[:, :],
                                 lhsT[:, k * N + m * P: k * N + m * P + P],
                                 rhs[:, k * N:(k + 1) * N],
                                 start=(i == 0), stop=(i == total - 1))
                i += 1
        pab.append(pm)
    return pab


def emit_gemm_1(nc, ps, name, mains, folds=()):
    """Like emit_gemm_sp but both m-blocks in ONE [P, 2N] psum tile (1 bank):
    staging is a single cheaper instruction; use for off-path GEMMs."""
    pm = ps.tile([P, 2 * N], F32, tag="ps2", name=name)
    total = 2 * (len(folds) + 2 * len(mains))
    i = 0
    for m in range(2):
        msl = slice(m * N, (m + 1) * N)
        for fl, fr in folds:
            nc.tensor.matmul(pm[:, msl],
                             fl[:, m * N + m * P: m * N + m * P + P],
                             fr[:, msl],
                             start=(i == 0), stop=(i == total - 1))
            i += 1
        for lhsT, rhs in mains:
            for k in range(2):
                nc.tensor.matmul(pm[:, msl],
                                 lhsT[:, k * N + m * P: k * N + m * P + P],
                                 rhs[:, k * N:(k + 1) * N],
                                 start=(i == 0), stop=(i == total - 1))
                i += 1
    return pm


def build_program():
    c = cheb_log_coeffs(CHEB_A, CHEB_B, CHEB_DEG)
    ca_, cb_, c3, c7 = ps_chunks_u(c)
    al = 2.0 / (CHEB_B - CHEB_A)
    be = -(CHEB_B + CHEB_A) / (CHEB_B - CHEB_A)

    # gamma-shift: evaluate (T4 + g I)@H1 with g = a2/b2 so the a2*T2 fold
    # vanishes: p = T4g@H1 + T3@R37 + a0' I + a1' u  with ai' = ai - g*bi.
    g = float(ca_[2] / cb_[2])
    a0p = float(ca_[0] - g * cb_[0])
    a1p = float(ca_[1] - g * cb_[1])
    kap = (g - 1.0) / 2.0             # pT4 psum = T2@T2 + kap*I; T4g = 2*psum

    iden = np.eye(N, dtype=np.float64)
    consts = {
        "iden": iden,
        "ibe2": be * iden,            # u = al*psum + ibe2
        "ikap": kap * iden,           # fold for T4g
        "B0c": cb_[0] * iden,         # H1a = b1*u + B0c
        "A0c": a0p * iden,            # pL0 folds
        "A1c": a1p * iden,
        "R0c": (c3 - c7 * g) * iden,  # R37 = 2*c7*pT4' + (c3 - c7*g)*I
    }
    CONST_NAMES = list(consts)
    const_arr = np.concatenate(
        [to_tile(round_f32r(consts[k])) for k in CONST_NAMES], axis=1)
    b1, b2 = float(cb_[1]), float(cb_[2])

    nc = bacc.Bacc()
    f_in = nc.declare_dram_parameter("fs", [2, NSTEP, P, 2 * N], F32,
                                     isOutput=False)
    tv_in = nc.declare_dram_parameter("tv", [P, 2 * NSTEP], F32, isOutput=False)
    c_in = nc.declare_dram_parameter("consts", [P, 2 * N * len(CONST_NAMES)],
                                     F32, isOutput=False)
    m_out = nc.declare_dram_parameter("means", [2, L_KEEP, P, 2 * N], F32,
                                      isOutput=True)

    with TileContext(nc) as tc:
        with (
            tc.tile_pool(name="consts", bufs=1) as cpool,
            tc.tile_pool(name="state", bufs=4) as spool,
            tc.tile_pool(name="work", bufs=2) as wpool,
            tc.tile_pool(name="fin", bufs=4) as fpool,
            tc.tile_pool(name="mout", bufs=2) as opool,
            tc.tile_pool(name="ps", bufs=6, space="PSUM") as ps,
            tc.tile_pool(name="ps2", bufs=2, space="PSUM") as ps2,
        ):
            CT = cpool.tile([P, 2 * N * len(CONST_NAMES)], F32R, tag="cc")
            nc.sync.dma_start(CT[:, :], c_in[:, :].bitcast(F32R))
            cv = {k: CT[:, i * 2 * N:(i + 1) * 2 * N]
                  for i, k in enumerate(CONST_NAMES)}
            TV = cpool.tile([P, 2 * NSTEP], F32, tag="tv")
            nc.sync.dma_start(TV[:, :], tv_in[:, :])

            IDEN = cv["iden"]
            Zt = [None, None]
            Z = [None, None]
            Ct = [None, None]

            def st(x):
                return IDEN if x is None else x[:, :]

            def act_halves(out, pab, scale=1.0):
                """scaled copy of a split-psum pair into tile halves (ACT)."""
                for h in range(2):
                    nc.scalar.activation(out[:, h * N:(h + 1) * N],
                                         pab[h][:, :], ACTF.Copy, scale=scale)

            def dve_halves(out, pab, scalar, in1, op0, op1):
                for h in range(2):
                    nc.vector.scalar_tensor_tensor(
                        out[:, h * N:(h + 1) * N], pab[h][:, :], scalar,
                        in1[:, h * N:(h + 1) * N], op0=op0, op1=op1)

            def act_full(out, pab, scale=1.0):
                """single full-width scaled copy over a split-psum pair
                (the pair is two ring-adjacent banks; read each separately
                but in ONE instruction is not expressible - use two, so
                "full" here just means emitted as the cheaper unsplit pair
                order for off-path results)."""
                act_halves(out, pab, scale=scale)

            def dve_full(out, pab, scalar, in1, op0, op1):
                dve_halves(out, pab, scalar, in1, op0, op1)

            # Lockstep emission: for each step j, every op is emitted for
            # chain 0 then chain 1, so the in-order engine queues alternate
            # chains and one chain's dependency gaps are covered by the
            # other's work.
            def step_ops(j):
                V = [{}, {}]

                def w_tile(ch, nm, dt=F32R):
                    V[ch][nm] = wpool.tile([P, 2 * N], dt, tag=f"{nm}{ch}",
                                           name=f"{nm}{ch}_{j}")
                    return V[ch][nm]

                def op_f(ch, v):
                    v["fs"] = fpool.tile([P, 2 * N], F32R, tag=f"f{ch}",
                                         name=f"fs{ch}_{j}")
                    nc.sync.dma_start(v["fs"][:, :],
                                      f_in[ch, j, :, :].bitcast(F32R))

                def op_pW(ch, v):
                    v["pW"] = emit_gemm_sp(nc, ps, f"pW{ch}_{j}",
                                           [(v["fs"][:, :], st(Zt[ch]))])

                def op_Wt(ch, v):
                    act_halves(w_tile(ch, "Wt"), v["pW"])

                def op_pS(ch, v):
                    v["pS"] = emit_gemm_sp(nc, ps, f"pS{ch}_{j}",
                                           [(st(Zt[ch]), v["Wt"][:, :])])

                def op_u(ch, v):
                    dve_halves(w_tile(ch, "u"), v["pS"], float(al),
                               cv["ibe2"], ALU.mult, ALU.add)

                def op_pU2(ch, v):
                    v["pU2"] = emit_gemm_sp(nc, ps, f"pU2{ch}_{j}",
                                            [(v["u"][:, :], v["u"][:, :])])

                def op_T2(ch, v):
                    dve_halves(w_tile(ch, "T2"), v["pU2"], 2.0, IDEN,
                               ALU.mult, ALU.subtract)

                def op_pT3(ch, v):
                    v["pT3"] = emit_gemm_sp(nc, ps, f"pT3{ch}_{j}",
                                            [(v["T2"][:, :], v["u"][:, :])])

                def op_T3(ch, v):
                    dve_halves(w_tile(ch, "T3"), v["pT3"], 2.0, v["u"][:, :],
                               ALU.mult, ALU.subtract)

                def op_pT4(ch, v):
                    # psum' = T2@T2 + kap I;  T4g = 2*psum' (ACT), and
                    # R37 = 2*c7*psum' + (c3 - c7*g) I (DVE)
                    v["pT4"] = emit_gemm_1(nc, ps2, f"pT4{ch}_{j}",
                                           [(v["T2"][:, :], v["T2"][:, :])],
                                           folds=[(cv["ikap"], IDEN)])

                def op_T4(ch, v):
                    t = w_tile(ch, "T4")
                    nc.scalar.activation(t[:, :], v["pT4"][:, :], ACTF.Copy,
                                         scale=2.0)

                def op_R37(ch, v):
                    t = w_tile(ch, "R37")
                    nc.vector.scalar_tensor_tensor(
                        t[:, :], v["pT4"][:, :], 2.0 * c7, cv["R0c"],
                        op0=ALU.mult, op1=ALU.add)

                def op_H1a(ch, v):
                    t = w_tile(ch, "H1a")
                    nc.vector.scalar_tensor_tensor(
                        t[:, :], v["u"][:, :], b1, cv["B0c"],
                        op0=ALU.mult, op1=ALU.add)

                def op_H1(ch, v):
                    t = w_tile(ch, "H1")
                    nc.vector.scalar_tensor_tensor(
                        t[:, :], v["T2"][:, :], b2, v["H1a"][:, :],
                        op0=ALU.mult, op1=ALU.add)

                def op_pL0(ch, v):
                    v["pL0"] = emit_gemm_sp(
                        nc, ps, f"pL0{ch}_{j}",
                        [(v["T3"][:, :], v["R37"][:, :]),
                         (v["T4"][:, :], v["H1"][:, :])],
                        folds=[(cv["A0c"], IDEN), (cv["A1c"], v["u"][:, :])])

                def op_X(ch, v):
                    tvc = TV[:, ch * NSTEP + j: ch * NSTEP + j + 1]
                    act_halves(w_tile(ch, "X"), v["pL0"], scale=tvc)

                def op_pX2(ch, v):
                    v["pX2"] = emit_gemm_sp(nc, ps, f"pX2{ch}_{j}",
                                            [(v["X"][:, :], v["X"][:, :])])

                def op_Shi(ch, v):
                    dve_halves(w_tile(ch, "Shi"), v["pX2"], float(1 / 6),
                               IDEN, ALU.mult, ALU.add)

                def op_Chm(ch, v):
                    act_halves(w_tile(ch, "Chm"), v["pX2"], scale=0.5)

                def op_pSh(ch, v):
                    v["pSh"] = emit_gemm_sp(nc, ps, f"pSh{ch}_{j}",
                                            [(v["X"][:, :], v["Shi"][:, :])])

                def op_Dm(ch, v):
                    # on DVE straight from PSUM: next step's Zt needs it
                    dve_halves(w_tile(ch, "Dm"), v["pSh"], -1.0,
                               v["Chm"][:, :], ALU.mult, ALU.add)

                def op_Dp(ch, v):
                    # Dp = Sh + Chm = 2*Chm - Dm (no Sh staging needed)
                    t = w_tile(ch, "Dp")
                    nc.vector.scalar_tensor_tensor(
                        t[:, :], v["Chm"][:, :], 2.0, v["Dm"][:, :],
                        op0=ALU.mult, op1=ALU.subtract)

                def op_pZt(ch, v):
                    v["pZt"] = emit_gemm_sp(nc, ps, f"pZt{ch}_{j}",
                                            [(st(Z[ch]), v["Dm"][:, :])],
                                            folds=[(IDEN, st(Zt[ch]))])

                def op_Ztn(ch, v):
                    v["Ztn"] = spool.tile([P, 2 * N], F32R, tag=f"Zt{ch}",
                                          name=f"Ztn{ch}_{j}")
                    act_halves(v["Ztn"], v["pZt"])

                def op_pZ(ch, v):
                    v["pZ"] = emit_gemm_1(nc, ps2, f"pZ{ch}_{j}",
                                          [(v["Dm"][:, :], st(Z[ch]))],
                                          folds=[(IDEN, st(Z[ch]))])

                def op_Zn(ch, v):
                    v["Zn"] = spool.tile([P, 2 * N], F32R, tag=f"Z{ch}",
                                         name=f"Zn{ch}_{j}")
                    nc.scalar.activation(v["Zn"][:, :], v["pZ"][:, :],
                                         ACTF.Copy)

                def op_pCt(ch, v):
                    v["pCt"] = emit_gemm_1(nc, ps2, f"pCt{ch}_{j}",
                                           [(v["Dp"][:, :], st(Ct[ch]))],
                                           folds=[(IDEN, st(Ct[ch]))])

                def op_Ctn(ch, v):
                    v["Ctn"] = spool.tile([P, 2 * N], F32R, tag=f"Ct{ch}",
                                          name=f"Ctn{ch}_{j}")
                    nc.scalar.activation(v["Ctn"][:, :], v["pCt"][:, :],
                                         ACTF.Copy)

                def op_out_prev(ch, v):
                    # output of step j-1 (committed state), deferred here so
                    # its GEMM fills gaps instead of stalling the PE tail
                    if j - 1 < W_WARM or Ct[ch] is None:
                        return
                    pM = emit_gemm_1(nc, ps2, f"pM{ch}_{j - 1}",
                                     [(st(Ct[ch]), st(Ct[ch]))])
                    Mo = opool.tile([P, 2 * N], F32, tag=f"Mo{ch}",
                                    name=f"Mo{ch}_{j - 1}")
                    nc.scalar.activation(Mo[:, :], pM[:, :], ACTF.Copy)
                    nc.sync.dma_start(m_out[ch, j - 1 - W_WARM, :, :],
                                      Mo[:, :])

                OPS1 = (op_f, op_out_prev, op_pW, op_Wt, op_pS, op_u,
                        op_pU2, op_T2, op_pT3, op_T3, op_pT4, op_T4,
                        op_R37, op_H1a, op_H1, op_pL0, op_X)
                OPS2 = (op_pX2, op_Shi, op_Chm, op_pSh, op_Dm,
                        op_Dp, op_pZt, op_Ztn, op_pZ, op_Zn, op_pCt,
                        op_Ctn)
                yield OPS1, OPS2, V


            # phase-shifted 2-chain schedule: per j emit
            #   A-phase1(j), B-phase2(j-1), B-phase1(j), A-phase2(j)
            # so each chain's mid-step dependency gap is covered by the
            # other chain's independent work.
            gens = {}
            phases = {}
            commit = {}

            def start(jj):
                gen = step_ops(jj)
                o1, o2, vv = next(gen)
                gens[jj] = (o1, o2, vv)

            def zip_phases(a_ops, a_v, b_ops, b_v):
                na, nb = len(a_ops or ()), len(b_ops or ())
                for i in range(max(na, nb)):
                    if i < nb:
                        b_ops[i](1, b_v)
                    if i < na:
                        a_ops[i](0, a_v)

            def commit(jj, ch):
                vv = gens[jj][2][ch]
                Zt[ch], Z[ch], Ct[ch] = vv["Ztn"], vv["Zn"], vv["Ctn"]

            for j in range(NSTEP + 1):
                if j < NSTEP:
                    start(j)
                ph_a1 = gens[j][0] if j < NSTEP else None
                av = gens[j][2][0] if j < NSTEP else None
                ph_b2 = gens[j - 1][1] if j >= 1 else None
                bv = gens[j - 1][2][1] if j >= 1 else None
                zip_phases(ph_a1 or (), av, ph_b2 or (), bv)
                if j >= 1:
                    commit(j - 1, 1)
                if j < NSTEP:
                    ph_a2, vv = gens[j][1], gens[j][2]
                    ph_b1 = gens[j][0]
                    na, nb = len(ph_a2), len(ph_b1)
                    for i in range(max(na, nb)):
                        if i < na:
                            ph_a2[i](0, vv[0])
                        if i < nb:
                            ph_b1[i](1, vv[1])
                    commit(j, 0)
                if j >= 1:
                    del gens[j - 1]

            # final kept outputs (state of the last step, never emitted by
            # the deferred op_out_prev)
            for ch in (0, 1):
                pM = emit_gemm_1(nc, ps2, f"pMfin{ch}",
                                 [(Ct[ch][:, :], Ct[ch][:, :])])
                Mo = opool.tile([P, 2 * N], F32, tag=f"Mo{ch}",
                                name=f"Mofin{ch}")
                nc.scalar.activation(Mo[:, :], pM[:, :], ACTF.Copy)
                nc.sync.dma_start(m_out[ch, L_KEEP - 1, :, :], Mo[:, :])

    nc.compile()
    return nc, const_arr


_CACHED = {}


def prep_inputs(f, weights, const_arr):
    f = np.asarray(f, dtype=np.float32)
    weights = np.asarray(weights, dtype=np.float32)
    fs = f[:, 0]
    e = np.exp(weights - weights.max(axis=1, keepdims=True))
    t = (e / e.sum(axis=1, keepdims=True))[:, 1].astype(np.float32)

    iden = np.eye(N, dtype=np.float32)
    f_tiles = np.empty((B + W_WARM, P, 2 * N), np.float32)
    f_tiles[:W_WARM] = to_tile(iden)
    for k in range(B):
        f_tiles[W_WARM + k] = to_tile(round_f32r(fs[k]))
    t_pad = np.concatenate([np.zeros(W_WARM, np.float32), t])

    in_maps = []
    for c in range(NCORES):
        fa = np.empty((2, NSTEP, P, 2 * N), np.float32)
        tva = np.empty((P, 2 * NSTEP), np.float32)
        for ch in range(2):
            w = 2 * c + ch
            s0 = w * L_KEEP           # window start in padded idx
            fa[ch] = f_tiles[s0:s0 + NSTEP]
            tva[:, ch * NSTEP:(ch + 1) * NSTEP] = np.broadcast_to(
                0.5 * t_pad[s0:s0 + NSTEP], (P, NSTEP))
        in_maps.append({"fs": np.ascontiguousarray(fa),
                        "tv": np.ascontiguousarray(tva),
                        "consts": const_arr})
    return in_maps


def unpack_outputs(results):
    out = np.empty((B, N, N), np.float32)
    for c in range(NCORES):
        m = results[c]["means"]       # [2, L_KEEP, P, 2N]
        for ch in range(2):
            w = 2 * c + ch
            for j in range(L_KEEP):
                out[w * L_KEEP + j] = from_tile(m[ch, j])
    return out[:, None]


def kernel(f, weights):
    if "prog" not in _CACHED:
        _CACHED["prog"] = build_program()
    nc, const_arr = _CACHED["prog"]
    in_maps = prep_inputs(f, weights, const_arr)
    res = run_bass_kernel_spmd(nc, in_maps, list(range(NCORES)))
    return unpack_outputs(res.results)
